# revision 5
# baseline (speedup 1.0000x reference)
"""Trainium2 Bass kernel for ContinualLoraMoeOneGateInjectedLinear.

Computation (see reference):
    route  = lora_route[task_id-1]            (or sum over tasks if task_id > 5)
    a      = x @ route                        [B,S,P]
    gate   = 2*mean(softmax(a, -1), S) - 1    [B,P]
    tid    = min(task_id, 5)
    delta  = sum_p gate[b,p] * (x @ down_p) @ up_p   (p < tid)
    y      = x @ linear_w.T + delta

Sharding: 8 cores = 4 batches x 2 token-halves.  Core k handles batch
k//2, tokens [2048*(k%2), 2048*(k%2+1)), full 1024-wide output.  The
routing/z pass runs once per token (not duplicated per output half as
in a batch x out-half split); the softmax token-mean needs the sibling
core's partial sums, exchanged with a 32 B pairwise AllReduce.

Device kernel (per core), heavy matmuls in float16 (fp32 accumulate):
  1. zaT[72, 2048] = [down|route].T @ x  (rank-40 LoRA-down + routing
     logits fused; route rows parked at partition 64), PE-transpose the
     logits token-major, softmax (max-free), ones-matmul partial sums
  2. partial[1,5] --AllReduce(pair)--> full sums -> gate[5] -> upeff
  3. y[2048, 1024] = x @ Wt in 32 half-groups of [128 tok, 512 out];
     W's two halves DMA separately so A-half groups can start before wB
     lands; the LoRA delta rides as a 9th accumulation per half for
     halves emitted after the gate (first UNFUSED A-halves get a
     deferred delta + DVE add); per-half 256 KB output DMAs drain
     continuously.
"""

import sys

if "/opt/trn_rl_repo" not in sys.path:
    sys.path.insert(0, "/opt/trn_rl_repo")

from contextlib import ExitStack

import numpy as np

import concourse.bass as bass
import concourse.mybir as mybir
import concourse.tile as tile
from concourse import bacc
from concourse.bass_utils import run_bass_kernel_spmd

F32 = mybir.dt.float32
F16 = mybir.dt.float16

NUM_TASKS = 5
B, S, IN, OUT, P, R = 4, 4096, 1024, 1024, 5, 8
RT = P * R  # 40 total low-rank dims
ZA = 72  # fused [down|route] matmul rows: 0:40 down, 64:69 route
RB = 64  # partition base of the route rows
SL = S // 2  # 2048 local tokens per core
NI = IN // 128  # 8 i-tiles
NC = SL // 512  # 4 token chunks of 512
NG = SL // 128  # 16 token tiles of 128
OH = OUT // 2  # 512-wide output half (one PSUM bank)

UNFUSED = 5  # A-half groups drained before the gate is ready


def build_kernel():
    """Build the per-core Bacc module (identical program on all 8 cores)."""
    nc = bacc.Bacc("TRN2", target_bir_lowering=False, debug=False, num_devices=8)

    xt_d = nc.dram_tensor("xt", [128, NC, NI * 512], F16, kind="ExternalInput").ap()
    wa_d = nc.dram_tensor("wa", [128, NI * OH], F16, kind="ExternalInput").ap()
    wb_d = nc.dram_tensor("wb", [128, NI * OH], F16, kind="ExternalInput").ap()
    rd_d = nc.dram_tensor("rd", [128, NI * ZA], F16, kind="ExternalInput").ap()
    up_d = nc.dram_tensor("up", [RT, OUT], F32, kind="ExternalInput").ap()
    eye_d = nc.dram_tensor("eye5", [P, P], F32, kind="ExternalInput").ap()
    ones_d = nc.dram_tensor("ones", [128, 1], F32, kind="ExternalInput").ap()
    e40_d = nc.dram_tensor("e40", [P, RT], F32, kind="ExternalInput").ap()
    y_d = nc.dram_tensor("y", [SL, OUT], F32, kind="ExternalOutput").ap()
    ccin_d = nc.dram_tensor("ccin", [1, 8], F32).ap()
    ccout_d = nc.dram_tensor("ccout", [1, 8], F32).ap()

    with tile.TileContext(nc) as tc, ExitStack() as ctx:
        consts = ctx.enter_context(tc.tile_pool(name="consts", bufs=1))
        rdp = ctx.enter_context(tc.tile_pool(name="rdp", bufs=1))
        wp = ctx.enter_context(tc.tile_pool(name="wp", bufs=1))
        xp = ctx.enter_context(tc.tile_pool(name="xp", bufs=NC))
        zp = ctx.enter_context(tc.tile_pool(name="zp", bufs=1))
        sfx = ctx.enter_context(tc.tile_pool(name="sfx", bufs=1))
        yb = ctx.enter_context(tc.tile_pool(name="yb", bufs=10))
        za_ps = ctx.enter_context(tc.tile_pool(name="za_ps", bufs=2, space="PSUM"))
        y_ps = ctx.enter_context(tc.tile_pool(name="y_ps", bufs=4, space="PSUM"))
        tr_ps = ctx.enter_context(tc.tile_pool(name="tr_ps", bufs=1, space="PSUM"))
        sm_ps = ctx.enter_context(tc.tile_pool(name="sm_ps", bufs=1, space="PSUM"))

        # HAM warmup on a memset tile: the PE clock-gate needs ~3.4us of
        # activity to reach 2.4 GHz; run junk matmuls while the DMAs land.
        junk = consts.tile([128, 128], F16)
        nc.gpsimd.memset(junk[:], 0.0)
        wps = tr_ps.tile([128, P * NG], F32, tag="trp")
        for _ in range(24):
            nc.tensor.matmul(
                wps[:], junk[:], junk[:, 0 : P * NG], start=True, stop=True
            )

        # input DMAs, ordered so za's x chunks land before wB
        rd_sb = rdp.tile([128, NI * ZA], F16)
        nc.sync.dma_start(rd_sb[:], rd_d)
        xt_t = {}

        def load_chunk(c):
            t = xp.tile([128, NI * 512], F16, tag="xt_t")
            h = NI * 256
            nc.sync.dma_start(t[:, 0:h], xt_d[:, c, 0:h])
            nc.sync.dma_start(t[:, h:], xt_d[:, c, h:])
            xt_t[c] = t

        load_chunk(0)
        load_chunk(1)
        wa_sb = wp.tile([128, NI * OH], F16)
        nc.sync.dma_start(wa_sb[:], wa_d)
        load_chunk(2)
        load_chunk(3)
        wb_sb = wp.tile([128, NI * OH], F16)
        nc.sync.dma_start(wb_sb[:], wb_d)
        eye5 = consts.tile([P, P], F32)
        ones = consts.tile([128, 1], F32)
        e40 = consts.tile([P, RT], F32)
        up_sb = consts.tile([RT, OUT], F32)
        for t, d in [(eye5, eye_d), (ones, ones_d), (e40, e40_d), (up_sb, up_d)]:
            nc.sync.dma_start(t[:], d)

        # fused [down|route] matmul + per-chunk drain + routing transposes
        zt_sb = zp.tile([RT, SL], F16)  # z^T, feeds the delta matmul
        at_sb = zp.tile([P, SL], F32)  # routing logits a^T
        trp = tr_ps.tile([128, P * NG], F32, tag="trp")  # a, token-major

        def emit_za(c):
            za = za_ps.tile([ZA, 512], F32, tag="za")
            for i in range(NI):
                nc.tensor.matmul(
                    za[:],
                    rd_sb[:, ZA * i : ZA * (i + 1)],
                    xt_t[c][:, 512 * i : 512 * (i + 1)],
                    start=(i == 0),
                    stop=(i == NI - 1),
                )
            nc.vector.tensor_copy(zt_sb[:, 512 * c : 512 * (c + 1)], za[0:RT, :])
            nc.scalar.copy(at_sb[:, 512 * c : 512 * (c + 1)], za[RB : RB + P, :])
            for q in range(4):
                g = 4 * c + q
                nc.tensor.transpose(
                    trp[:, P * g : P * (g + 1)],
                    at_sb[:, 128 * g : 128 * (g + 1)],
                    eye5[:],
                )

        # main y = x @ W matmuls, in 512-wide half-groups (one PSUM bank).
        # half 0 uses wa_sb (lands before wb_sb).  Fused halves take the
        # delta matmul as a 9th accumulation; unfused ones get a deferred
        # delta matmul + DVE add before their output DMA.
        upeff = {}

        def emit_half(g, half, fused):
            w_sb = wa_sb if half == 0 else wb_sb
            c, q = g // 4, g % 4
            ypt = y_ps.tile([128, OH], F32, tag="ypt")
            for i in range(NI):
                nc.tensor.matmul(
                    ypt[:],
                    xt_t[c][:, 512 * i + 128 * q : 512 * i + 128 * (q + 1)],
                    w_sb[:, OH * i : OH * (i + 1)],
                    start=(i == 0),
                    stop=(i == NI - 1) and not fused,
                )
            if fused:
                nc.tensor.matmul(
                    ypt[:],
                    zt_sb[:, 128 * g : 128 * (g + 1)],
                    upeff[half][:],
                    start=False,
                    stop=True,
                )
            yt = yb.tile([128, OH], F32, tag="yst")
            if (g + half) % 2 == 0:
                nc.vector.tensor_copy(yt[:], ypt[:])
            else:
                nc.scalar.copy(yt[:], ypt[:])
            if fused:
                dma_half(g, half, yt)
            return yt

        def dma_half(g, half, yt):
            nc.sync.dma_start(y_d[128 * g : 128 * (g + 1), OH * half : OH * (half + 1)], yt[:])

        def emit_deferred_delta(g, half, yt):
            dpt = y_ps.tile([128, OH], F32, tag="ypt")
            nc.tensor.matmul(
                dpt[:],
                zt_sb[:, 128 * g : 128 * (g + 1)],
                upeff[half][:],
                start=True,
                stop=True,
            )
            nc.vector.tensor_add(yt[:], yt[:], dpt[:])
            dma_half(g, half, yt)

        # softmax over experts (max-free: |a| < ~4) and token partial sums,
        # then the pairwise AllReduce and the gate -> upeff chain
        def emit_gate():
            e_sb = sfx.tile([128, P * NG], F32)
            nc.scalar.activation(e_sb[:], trp[:], mybir.ActivationFunctionType.Exp)
            den = sfx.tile([128, NG], F32)
            nc.vector.tensor_reduce(
                den[:],
                e_sb[:].rearrange("p (g f) -> p g f", f=P),
                axis=mybir.AxisListType.X,
                op=mybir.AluOpType.add,
            )
            invd = sfx.tile([128, NG], F32)
            nc.vector.reciprocal(invd[:], den[:])
            om = sfx.tile([128, P * NG], F32)
            nc.vector.tensor_tensor(
                om[:].rearrange("p (g f) -> p g f", f=P),
                e_sb[:].rearrange("p (g f) -> p g f", f=P),
                invd[:].unsqueeze(2).to_broadcast((128, NG, P)),
                mybir.AluOpType.mult,
            )
            # token partial sums: one matmul -> [1, 80], then strided reduce
            pp = sm_ps.tile([1, P * NG], F32, tag="sm")
            nc.tensor.matmul(pp[:], ones[:, 0:1], om[:], start=True, stop=True)
            part8 = sfx.tile([1, 8], F32)
            nc.gpsimd.memset(part8[:], 0.0)
            nc.vector.tensor_reduce(
                part8[0:1, 0:P],
                pp[:].rearrange("p (g f) -> p f g", f=P),
                axis=mybir.AxisListType.X,
                op=mybir.AluOpType.add,
            )
            # pairwise AllReduce of the partial sums (32 B)
            nc.sync.dma_start(ccin_d, part8[:])
            nc.gpsimd.collective_compute(
                "AllReduce",
                mybir.AluOpType.add,
                replica_groups=[[0, 1], [2, 3], [4, 5], [6, 7]],
                ins=[ccin_d],
                outs=[ccout_d],
            )
            gsum = sfx.tile([1, 8], F32)
            nc.sync.dma_start(gsum[:], ccout_d)
            # gate = 2/S * sum - 1, still as a row [1, 5]
            grow2 = sfx.tile([1, P], F32)
            nc.scalar.activation(
                grow2[:],
                gsum[0:1, 0:P],
                mybir.ActivationFunctionType.Copy,
                bias=-1.0,
                scale=2.0 / S,
            )
            gp = sm_ps.tile([P, 1], F32, tag="sm")
            nc.tensor.transpose(gp[:], grow2[:], eye5[0:1, 0:1])
            g5 = sfx.tile([P, 1], F32)
            nc.any.tensor_copy(g5[:], gp[:])
            ep = sm_ps.tile([RT, 1], F32, tag="sm")
            nc.tensor.matmul(ep[:], e40[:], g5[:], start=True, stop=True)
            g40 = sfx.tile([RT, 1], F32)
            nc.any.tensor_copy(g40[:], ep[:])
            for half in range(2):
                ueff = sfx.tile([RT, OH], F16)
                nc.vector.tensor_scalar_mul(
                    ueff[:], up_sb[:, OH * half : OH * (half + 1)], g40[:]
                )
                upeff[half] = ueff

        # emission order ~ execution order: za chunks paced by x DMA with
        # A-halves as filler, gate chain, then the rest of the halves with
        # the deferred deltas for the first UNFUSED A-halves interleaved.
        emit_za(0)
        emit_za(1)
        a_stage = {}
        a_stage[0] = emit_half(0, 0, fused=False)
        emit_za(2)
        a_stage[1] = emit_half(1, 0, fused=False)
        emit_za(3)
        emit_gate()
        for g in range(2, UNFUSED):
            a_stage[g] = emit_half(g, 0, fused=False)
        for g in range(UNFUSED, NG):
            emit_half(g, 0, fused=True)
            if g - UNFUSED < UNFUSED:
                emit_deferred_delta(g - UNFUSED, 0, a_stage[g - UNFUSED])
        for g in range(NG):
            emit_half(g, 1, fused=True)

    nc.compile()
    return nc


def _host_prep(inputs):
    """Shard/transform full inputs into the 8 per-core input maps."""
    x = np.asarray(inputs["input"], dtype=np.float32).reshape(B, S, IN)
    linear_w = np.asarray(inputs["linear_w"], dtype=np.float32)
    lora_down = np.asarray(inputs["lora_down"], dtype=np.float32)
    lora_up = np.asarray(inputs["lora_up"], dtype=np.float32)
    lora_route = np.asarray(inputs["lora_route"], dtype=np.float32)
    task_id = int(np.asarray(inputs["task_id"]))

    if task_id <= NUM_TASKS:
        route = lora_route[task_id - 1]  # python negative-index semantics
    else:
        route = lora_route.sum(axis=0)
    tid = min(task_id, NUM_TASKS)

    up_cat = np.zeros((RT, OUT), dtype=np.float32)
    rd = np.zeros((IN, ZA), dtype=np.float32)  # [down | pad | route]
    for p in range(tid):
        rd[:, p * R : (p + 1) * R] = lora_down[p]
        up_cat[p * R : (p + 1) * R, :] = lora_up[p]
    rd[:, RB : RB + P] = route
    rd = np.ascontiguousarray(
        rd.astype(np.float16).reshape(NI, 128, ZA).transpose(1, 0, 2).reshape(128, NI * ZA)
    )
    wt = np.ascontiguousarray(linear_w.T)  # [IN, OUT]
    eye5 = np.eye(P, dtype=np.float32)
    ones = np.ones((128, 1), dtype=np.float32)
    e40 = np.zeros((P, RT), dtype=np.float32)
    for p in range(P):
        e40[p, p * R : (p + 1) * R] = 1.0

    # x^T chunk-fused layout per core: [128, NC, NI*512]
    xts = []
    for b in range(B):
        for half in range(2):
            xs = x[b, half * SL : (half + 1) * SL]
            xtb = xs.T.astype(np.float16).reshape(NI, 128, NC, 512)
            xts.append(
                np.ascontiguousarray(xtb.transpose(1, 2, 0, 3).reshape(128, NC, NI * 512))
            )
    ws = []
    for h in range(2):
        wh = wt[:, h * OH : (h + 1) * OH].astype(np.float16).reshape(NI, 128, OH)
        ws.append(np.ascontiguousarray(wh.transpose(1, 0, 2).reshape(128, NI * OH)))

    in_maps = []
    for k in range(8):
        in_maps.append(
            {
                "xt": xts[k],
                "wa": ws[0],
                "wb": ws[1],
                "rd": rd,
                "up": up_cat,
                "eye5": eye5,
                "ones": ones,
                "e40": e40,
            }
        )
    return in_maps


def _assemble(results):
    out = np.empty((B, S, OUT), dtype=np.float32)
    for k in range(8):
        b, half = k // 2, k % 2
        out[b, half * SL : (half + 1) * SL, :] = results[k]["y"]
    return out


def kernel(**inputs) -> np.ndarray:
    nc = build_kernel()
    in_maps = _host_prep(inputs)
    res = run_bass_kernel_spmd(nc, in_maps, core_ids=list(range(8)))
    return _assemble(res.results)


if __name__ == "__main__":
    rng = np.random.default_rng(0)
    demo = {
        "input": rng.standard_normal((B, S, IN), dtype=np.float32),
        "linear_w": (rng.standard_normal((OUT, IN)) * 0.02).astype(np.float32),
        "lora_down": (rng.standard_normal((P, IN, R)) * 0.02).astype(np.float32),
        "lora_up": (rng.standard_normal((P, R, OUT)) * 0.02).astype(np.float32),
        "lora_route": (rng.standard_normal((P, IN, P)) * 0.02).astype(np.float32),
        "task_id": 5,
    }
    y = kernel(**demo)
    print("ok", y.shape, y.dtype)


# revision 7
# speedup vs baseline: 1.1067x; 1.1067x over previous
"""Trainium2 Bass kernel for ContinualLoraMoeOneGateInjectedLinear.

Computation (see reference):
    route  = lora_route[task_id-1]            (or sum over tasks if task_id > 5)
    a      = x @ route                        [B,S,P]
    gate   = 2*mean(softmax(a, -1), S) - 1    [B,P]
    tid    = min(task_id, 5)
    delta  = sum_p gate[b,p] * (x @ down_p) @ up_p   (p < tid)
    y      = x @ linear_w.T + delta

Sharding: 8 cores = 4 batches x 2 token-halves.  Core k handles batch
k//2, tokens [2048*(k%2), 2048*(k%2+1)), full 1024-wide output.  The
routing/z pass runs once per token (not duplicated per output half as
in a batch x out-half split); the softmax token-mean needs the sibling
core's partial sums, exchanged with a 32 B pairwise AllReduce.  The
AllReduce pipeline is ~22us deep, so it is triggered as early as
possible (right after the routing pass, ~24us in) and its latency is
hidden under gate-independent base matmuls: all 16 A-halves (and the
first B-halves) run base-only and take a deferred delta + DVE add.

Device kernel (per core), heavy matmuls in float16 (fp32 accumulate):
  1. zaT[72, 2048] = [down|route].T @ x  (rank-40 LoRA-down + routing
     logits fused; route rows parked at partition 64), PE-transpose the
     logits token-major, softmax (max-free), ones-matmul partial sums
  2. partial[1,5] --AllReduce(pair)--> full sums -> gate[5] -> upeff
  3. y[2048, 1024] = x @ Wt in 32 half-groups of [128 tok, 512 out];
     fused halves take the delta matmul as a 9th accumulation; output
     staged to fp16 (halves the output DMA) and DMA'd per half.
"""

import sys

if "/opt/trn_rl_repo" not in sys.path:
    sys.path.insert(0, "/opt/trn_rl_repo")

from contextlib import ExitStack

import numpy as np

import concourse.bass as bass
import concourse.mybir as mybir
import concourse.tile as tile
from concourse import bacc
from concourse.bass_utils import run_bass_kernel_spmd

F32 = mybir.dt.float32
F16 = mybir.dt.float16

NUM_TASKS = 5
B, S, IN, OUT, P, R = 4, 4096, 1024, 1024, 5, 8
RT = P * R  # 40 total low-rank dims
ZA = 72  # fused [down|route] matmul rows: 0:40 down, 64:69 route
RB = 64  # partition base of the route rows
SL = S // 2  # 2048 local tokens per core
NI = IN // 128  # 8 i-tiles
NC = SL // 512  # 4 token chunks of 512
NG = SL // 128  # 16 token tiles of 128
OH = OUT // 2  # 512-wide output half (one PSUM bank)

UNFUSED_B = 2  # B-halves kept gate-independent as cc-latency margin


def build_kernel():
    """Build the per-core Bacc module (identical program on all 8 cores)."""
    nc = bacc.Bacc("TRN2", target_bir_lowering=False, debug=False, num_devices=8)

    xt_d = nc.dram_tensor("xt", [128, NC, NI * 512], F16, kind="ExternalInput").ap()
    wa_d = nc.dram_tensor("wa", [128, NI * OH], F16, kind="ExternalInput").ap()
    wb_d = nc.dram_tensor("wb", [128, NI * OH], F16, kind="ExternalInput").ap()
    rd_d = nc.dram_tensor("rd", [128, NI * ZA], F16, kind="ExternalInput").ap()
    up_d = nc.dram_tensor("up", [RT, OUT], F32, kind="ExternalInput").ap()
    eye_d = nc.dram_tensor("eye5", [P, P], F32, kind="ExternalInput").ap()
    ones_d = nc.dram_tensor("ones", [128, 1], F32, kind="ExternalInput").ap()
    e40_d = nc.dram_tensor("e40", [P, RT], F32, kind="ExternalInput").ap()
    y_d = nc.dram_tensor("y", [SL, OUT], F16, kind="ExternalOutput").ap()
    ccin_d = nc.dram_tensor("ccin", [1, 8], F32).ap()
    ccout_d = nc.dram_tensor("ccout", [1, 8], F32).ap()

    with tile.TileContext(nc) as tc, ExitStack() as ctx:
        consts = ctx.enter_context(tc.tile_pool(name="consts", bufs=1))
        rdp = ctx.enter_context(tc.tile_pool(name="rdp", bufs=1))
        wp = ctx.enter_context(tc.tile_pool(name="wp", bufs=1))
        xp = ctx.enter_context(tc.tile_pool(name="xp", bufs=NC))
        zp = ctx.enter_context(tc.tile_pool(name="zp", bufs=1))
        sfx = ctx.enter_context(tc.tile_pool(name="sfx", bufs=1))
        yb = ctx.enter_context(tc.tile_pool(name="yb", bufs=20))
        za_ps = ctx.enter_context(tc.tile_pool(name="za_ps", bufs=2, space="PSUM"))
        y_ps = ctx.enter_context(tc.tile_pool(name="y_ps", bufs=4, space="PSUM"))
        tr_ps = ctx.enter_context(tc.tile_pool(name="tr_ps", bufs=1, space="PSUM"))
        sm_ps = ctx.enter_context(tc.tile_pool(name="sm_ps", bufs=1, space="PSUM"))

        # HAM warmup + early-DMA filler on a memset tile: keeps the PE busy
        # (and its clock-gate at 2.4 GHz) until the first x chunk lands.
        junk = consts.tile([128, 128], F16)
        nc.gpsimd.memset(junk[:], 0.0)
        part8 = sfx.tile([1, 8], F32)
        nc.gpsimd.memset(part8[:], 0.0)
        wps = tr_ps.tile([128, P * NG], F32, tag="trp")
        for _ in range(56):
            nc.tensor.matmul(wps[:], junk[:], junk[:, 0 : P * NG], start=True, stop=True)

        # input DMAs: x first (the routing pass gates the AllReduce), W after
        rd_sb = rdp.tile([128, NI * ZA], F16)
        nc.sync.dma_start(rd_sb[:], rd_d)
        xt_t = {}

        def load_chunk(c):
            t = xp.tile([128, NI * 512], F16, tag="xt_t")
            h = NI * 256
            nc.sync.dma_start(t[:, 0:h], xt_d[:, c, 0:h])
            nc.sync.dma_start(t[:, h:], xt_d[:, c, h:])
            xt_t[c] = t

        for c in range(NC):
            load_chunk(c)
        wa_sb = wp.tile([128, NI * OH], F16)
        nc.sync.dma_start(wa_sb[:], wa_d)
        wb_sb = wp.tile([128, NI * OH], F16)
        nc.sync.dma_start(wb_sb[:], wb_d)
        eye5 = consts.tile([P, P], F32)
        ones = consts.tile([128, 1], F32)
        e40 = consts.tile([P, RT], F32)
        up_sb = consts.tile([RT, OUT], F32)
        for t, d in [(eye5, eye_d), (ones, ones_d), (e40, e40_d), (up_sb, up_d)]:
            nc.sync.dma_start(t[:], d)

        # fused [down|route] matmul + per-chunk drain + routing transposes
        zt_sb = zp.tile([RT, SL], F16)  # z^T, feeds the delta matmul
        at_sb = zp.tile([P, SL], F32)  # routing logits a^T
        trp = tr_ps.tile([128, P * NG], F32, tag="trp")  # a, token-major

        def emit_za(c):
            za = za_ps.tile([ZA, 512], F32, tag="za")
            for i in range(NI):
                nc.tensor.matmul(
                    za[:],
                    rd_sb[:, ZA * i : ZA * (i + 1)],
                    xt_t[c][:, 512 * i : 512 * (i + 1)],
                    start=(i == 0),
                    stop=(i == NI - 1),
                )
            nc.vector.tensor_copy(zt_sb[:, 512 * c : 512 * (c + 1)], za[0:RT, :])
            nc.scalar.copy(at_sb[:, 512 * c : 512 * (c + 1)], za[RB : RB + P, :])
            for q in range(4):
                g = 4 * c + q
                nc.tensor.transpose(
                    trp[:, P * g : P * (g + 1)],
                    at_sb[:, 128 * g : 128 * (g + 1)],
                    eye5[:],
                )

        for c in range(NC):
            emit_za(c)

        # softmax over experts (max-free: |a| < ~4), token partial sums,
        # pairwise AllReduce, then gate -> upeff
        upeff = {}
        e_sb = sfx.tile([128, P * NG], F32)
        nc.scalar.activation(e_sb[:], trp[:], mybir.ActivationFunctionType.Exp)
        den = sfx.tile([128, NG], F32)
        nc.vector.tensor_reduce(
            den[:],
            e_sb[:].rearrange("p (g f) -> p g f", f=P),
            axis=mybir.AxisListType.X,
            op=mybir.AluOpType.add,
        )
        invd = sfx.tile([128, NG], F32)
        nc.vector.reciprocal(invd[:], den[:])
        om = sfx.tile([128, P * NG], F32)
        nc.vector.tensor_tensor(
            om[:].rearrange("p (g f) -> p g f", f=P),
            e_sb[:].rearrange("p (g f) -> p g f", f=P),
            invd[:].unsqueeze(2).to_broadcast((128, NG, P)),
            mybir.AluOpType.mult,
        )
        pp = sm_ps.tile([1, P * NG], F32, tag="sm")
        nc.tensor.matmul(pp[:], ones[:, 0:1], om[:], start=True, stop=True)
        nc.vector.tensor_reduce(
            part8[0:1, 0:P],
            pp[:].rearrange("p (g f) -> p f g", f=P),
            axis=mybir.AxisListType.X,
            op=mybir.AluOpType.add,
        )
        nc.sync.dma_start(ccin_d, part8[:])
        nc.gpsimd.collective_compute(
            "AllReduce",
            mybir.AluOpType.add,
            replica_groups=[[0, 1], [2, 3], [4, 5], [6, 7]],
            ins=[ccin_d],
            outs=[ccout_d],
        )
        gsum = sfx.tile([1, 8], F32)
        nc.sync.dma_start(gsum[:], ccout_d)
        grow2 = sfx.tile([1, P], F32)
        nc.scalar.activation(
            grow2[:],
            gsum[0:1, 0:P],
            mybir.ActivationFunctionType.Copy,
            bias=-1.0,
            scale=2.0 / S,
        )
        gp = sm_ps.tile([P, 1], F32, tag="sm")
        nc.tensor.transpose(gp[:], grow2[:], eye5[0:1, 0:1])
        g5 = sfx.tile([P, 1], F32)
        nc.any.tensor_copy(g5[:], gp[:])
        ep = sm_ps.tile([RT, 1], F32, tag="sm")
        nc.tensor.matmul(ep[:], e40[:], g5[:], start=True, stop=True)
        g40 = sfx.tile([RT, 1], F32)
        nc.any.tensor_copy(g40[:], ep[:])
        for half in range(2):
            ueff = sfx.tile([RT, OH], F16, tag=f"ueff{half}")
            nc.vector.tensor_scalar_mul(
                ueff[:], up_sb[:, OH * half : OH * (half + 1)], g40[:]
            )
            upeff[half] = ueff

        # main y = x @ W matmuls, in 512-wide half-groups (one PSUM bank).
        # Fused halves take the delta matmul as a 9th accumulation; unfused
        # ones get a deferred delta matmul + DVE add before their DMA.
        def emit_half(g, half, fused):
            w_sb = wa_sb if half == 0 else wb_sb
            c, q = g // 4, g % 4
            ypt = y_ps.tile([128, OH], F32, tag="ypt")
            for i in range(NI):
                nc.tensor.matmul(
                    ypt[:],
                    xt_t[c][:, 512 * i + 128 * q : 512 * i + 128 * (q + 1)],
                    w_sb[:, OH * i : OH * (i + 1)],
                    start=(i == 0),
                    stop=(i == NI - 1) and not fused,
                )
            if fused:
                nc.tensor.matmul(
                    ypt[:],
                    zt_sb[:, 128 * g : 128 * (g + 1)],
                    upeff[half][:],
                    start=False,
                    stop=True,
                )
            yt = yb.tile([128, OH], F16, tag="yst")
            if (g + half) % 2 == 0:
                nc.vector.tensor_copy(yt[:], ypt[:])
            else:
                nc.scalar.copy(yt[:], ypt[:])
            if fused:
                dma_half(g, half, yt)
            return yt

        def dma_half(g, half, yt):
            nc.sync.dma_start(
                y_d[128 * g : 128 * (g + 1), OH * half : OH * (half + 1)], yt[:]
            )

        def emit_deferred_delta(g, half, yt):
            dpt = y_ps.tile([128, OH], F32, tag="ypt")
            nc.tensor.matmul(
                dpt[:],
                zt_sb[:, 128 * g : 128 * (g + 1)],
                upeff[half][:],
                start=True,
                stop=True,
            )
            nc.vector.tensor_add(yt[:], yt[:], dpt[:])
            dma_half(g, half, yt)

        # A-halves (and the first B-halves) run base-only while the
        # AllReduce is in flight; their deferred deltas interleave with the
        # fused B-halves once the gate lands.
        deferred = []
        for g in range(NG):
            deferred.append((g, 0, emit_half(g, 0, fused=False)))
        for g in range(UNFUSED_B):
            deferred.append((g, 1, emit_half(g, 1, fused=False)))
        for g in range(UNFUSED_B, NG):
            emit_half(g, 1, fused=True)
            if deferred:
                emit_deferred_delta(*deferred.pop(0))
        while deferred:
            emit_deferred_delta(*deferred.pop(0))

    nc.compile()
    return nc


def _host_prep(inputs):
    """Shard/transform full inputs into the 8 per-core input maps."""
    x = np.asarray(inputs["input"], dtype=np.float32).reshape(B, S, IN)
    linear_w = np.asarray(inputs["linear_w"], dtype=np.float32)
    lora_down = np.asarray(inputs["lora_down"], dtype=np.float32)
    lora_up = np.asarray(inputs["lora_up"], dtype=np.float32)
    lora_route = np.asarray(inputs["lora_route"], dtype=np.float32)
    task_id = int(np.asarray(inputs["task_id"]))

    if task_id <= NUM_TASKS:
        route = lora_route[task_id - 1]  # python negative-index semantics
    else:
        route = lora_route.sum(axis=0)
    tid = min(task_id, NUM_TASKS)

    up_cat = np.zeros((RT, OUT), dtype=np.float32)
    rd = np.zeros((IN, ZA), dtype=np.float32)  # [down | pad | route]
    for p in range(tid):
        rd[:, p * R : (p + 1) * R] = lora_down[p]
        up_cat[p * R : (p + 1) * R, :] = lora_up[p]
    rd[:, RB : RB + P] = route
    rd = np.ascontiguousarray(
        rd.astype(np.float16).reshape(NI, 128, ZA).transpose(1, 0, 2).reshape(128, NI * ZA)
    )
    wt = np.ascontiguousarray(linear_w.T)  # [IN, OUT]
    eye5 = np.eye(P, dtype=np.float32)
    ones = np.ones((128, 1), dtype=np.float32)
    e40 = np.zeros((P, RT), dtype=np.float32)
    for p in range(P):
        e40[p, p * R : (p + 1) * R] = 1.0

    # x^T chunk-fused layout per core: [128, NC, NI*512]
    xts = []
    for b in range(B):
        for half in range(2):
            xs = x[b, half * SL : (half + 1) * SL]
            xtb = xs.T.astype(np.float16).reshape(NI, 128, NC, 512)
            xts.append(
                np.ascontiguousarray(xtb.transpose(1, 2, 0, 3).reshape(128, NC, NI * 512))
            )
    ws = []
    for h in range(2):
        wh = wt[:, h * OH : (h + 1) * OH].astype(np.float16).reshape(NI, 128, OH)
        ws.append(np.ascontiguousarray(wh.transpose(1, 0, 2).reshape(128, NI * OH)))

    in_maps = []
    for k in range(8):
        in_maps.append(
            {
                "xt": xts[k],
                "wa": ws[0],
                "wb": ws[1],
                "rd": rd,
                "up": up_cat,
                "eye5": eye5,
                "ones": ones,
                "e40": e40,
            }
        )
    return in_maps


def _assemble(results):
    out = np.empty((B, S, OUT), dtype=np.float32)
    for k in range(8):
        b, half = k // 2, k % 2
        out[b, half * SL : (half + 1) * SL, :] = results[k]["y"].astype(np.float32)
    return out


def kernel(**inputs) -> np.ndarray:
    nc = build_kernel()
    in_maps = _host_prep(inputs)
    res = run_bass_kernel_spmd(nc, in_maps, core_ids=list(range(8)))
    return _assemble(res.results)


if __name__ == "__main__":
    rng = np.random.default_rng(0)
    demo = {
        "input": rng.standard_normal((B, S, IN), dtype=np.float32),
        "linear_w": (rng.standard_normal((OUT, IN)) * 0.02).astype(np.float32),
        "lora_down": (rng.standard_normal((P, IN, R)) * 0.02).astype(np.float32),
        "lora_up": (rng.standard_normal((P, R, OUT)) * 0.02).astype(np.float32),
        "lora_route": (rng.standard_normal((P, IN, P)) * 0.02).astype(np.float32),
        "task_id": 5,
    }
    y = kernel(**demo)
    print("ok", y.shape, y.dtype)
